# revision 17
# baseline (speedup 1.0000x reference)
"""CentroidLoss Trainium2 kernel (Bass/Tile, 8-core SPMD).

Math: the loss decomposes so embeddings are streamed exactly once.
  counts_c = #{n: t_n = c};  S_c = sum_{n in c} e_n;  centers c_c = S_c/(counts_c+eps)
  l2  = mean_n ||e_n - c_{t_n}||^2
      = ( sum_n ||e_n||^2 - 2 sum_c S_c.c_c + sum_c counts_c ||c_c||^2 ) / N
  cc  = sum_{a<b} relu(margin - sqrt(||c_a-c_b||^2 + eps))^2 * (C-1)/(2C)

Each core streams N/8 rows once: per-class sums+counts via one-hot matmuls
(bf16 operands, fp32 PSUM accumulation — the one-hot and the appended ones
column are exact in bf16, and the bf16 rounding of embeddings only perturbs
the tiny cross terms), and the dominant sum-of-squares term in full fp32 via
one ACT Square+accumulate per chunk.  The [C,130] partials are AllGathered
across the 8 cores, reduced, and the closed-form finale runs replicated.
"""

import numpy as np

try:
    import concourse.bass as bass
except ImportError:  # fallback when PYTHONPATH doesn't carry the repo
    import sys

    for _p in ("/opt/trn_rl_repo", "/root/.axon_site/_ro/trn_rl_repo"):
        if _p not in sys.path:
            sys.path.insert(0, _p)
    import concourse.bass as bass

from contextlib import ExitStack

import concourse.bacc as bacc
import concourse.tile as tile
from concourse import mybir
from concourse.bass_utils import run_bass_kernel_spmd

F32 = mybir.dt.float32
BF16 = mybir.dt.bfloat16
I32 = mybir.dt.int32
OP = mybir.AluOpType
AF = mybir.ActivationFunctionType

N_FULL = 262144
D = 128
C = 64
NCORES = 8
P = 128
MARGIN = 1.4
EPS = 1e-6

NLOC = N_FULL // NCORES  # rows per core
OHB = 8  # row-tiles of one-hot built per DVE instruction


def _build(nloc, chunk, oh_split=True):
    """Trace the SPMD kernel for `nloc` rows/core, `chunk` row-tiles per DMA."""
    T = nloc // P  # 128-row tiles per core
    assert chunk % OHB == 0
    assert nloc % P == 0 and T % chunk == 0
    nch = T // chunk
    nq = (T + P - 1) // P  # groups of <=128 tiles for the target transpose
    invN = 1.0 / (nloc * NCORES)
    ccs = (C - 1) / (2.0 * C)
    PKW = 132  # packed partial width: 0..127 sums, 128 counts, 129 ssq, pad

    nc = bacc.Bacc("TRN2", target_bir_lowering=False, debug=False, num_devices=NCORES)
    emb = nc.dram_tensor("emb", [nloc, D], F32, kind="ExternalInput")
    tgt = nc.dram_tensor("tgt", [nloc], I32, kind="ExternalInput")
    iota_in = nc.dram_tensor("iota_c", [P, OHB, C], BF16, kind="ExternalInput")
    eye_in = nc.dram_tensor("eye", [P, P], F32, kind="ExternalInput")
    triu_in = nc.dram_tensor("triu", [C, C], F32, kind="ExternalInput")
    out = nc.dram_tensor("out", [1, 1], F32, kind="ExternalOutput")

    with tile.TileContext(nc) as tc, ExitStack() as ctx:
        consts = ctx.enter_context(tc.tile_pool(name="consts", bufs=1))
        chunks = ctx.enter_context(tc.tile_pool(name="chunks", bufs=4))
        chunksb = ctx.enter_context(tc.tile_pool(name="chunksb", bufs=4))
        ohp = ctx.enter_context(tc.tile_pool(name="ohp", bufs=6))
        pacc = ctx.enter_context(tc.tile_pool(name="pacc", bufs=1, space="PSUM"))
        ptr = ctx.enter_context(tc.tile_pool(name="ptr", bufs=2, space="PSUM"))
        dram = ctx.enter_context(tc.tile_pool(name="dram", bufs=1, space="DRAM"))
        finp = ctx.enter_context(tc.tile_pool(name="finp", bufs=1))

        # constants
        iota_t = consts.tile([P, OHB, C], BF16)
        nc.sync.dma_start(iota_t[:], iota_in[:])
        eye_t = consts.tile([P, P], F32)
        nc.sync.dma_start(eye_t[:], eye_in[:])
        eye16 = consts.tile([P, P], BF16)
        nc.vector.tensor_copy(eye16[:], eye_t[:])
        triu_t = consts.tile([C, C], F32)
        nc.sync.dma_start(triu_t[:], triu_in[:])
        ones128 = consts.tile([P, 1], F32)
        nc.vector.memset(ones128[:], 1.0)
        ones_row = consts.tile([1, C], F32)
        nc.vector.memset(ones_row[:], 1.0)
        sc_invN = consts.tile([P, 1], F32)
        nc.vector.memset(sc_invN[:], invN)
        sc_ccs = consts.tile([C, 1], F32)
        nc.vector.memset(sc_ccs[:], ccs)
        eps_c = consts.tile([C, 1], F32)
        nc.vector.memset(eps_c[:], EPS)
        mar_c = consts.tile([C, 1], F32)
        nc.vector.memset(mar_c[:], MARGIN)
        zero_p = consts.tile([P, 1], F32)
        nc.vector.memset(zero_p[:], 0.0)

        # targets -> tgtT[P, T] bf16: column i holds the 128 targets of row-tile i
        tgt2 = tgt[:].rearrange("(t c) -> t c", c=P)  # [T, P]
        tgtT = consts.tile([P, T], BF16)
        for q in range(nq):
            rq = min(P, T - q * P)
            ti = consts.tile([P, P], I32, tag="tgt_i32")
            tf = consts.tile([P, P], BF16, tag="tgt_bf")
            nc.sync.dma_start(ti[:rq, :], tgt2[q * P : q * P + rq, :])
            nc.vector.tensor_copy(tf[:rq, :], ti[:rq, :])
            pt = ptr.tile([P, P], BF16, tag="pt")
            nc.tensor.transpose(pt[:, :rq], tf[:rq, :], eye16[:rq, :rq])
            nc.scalar.copy(tgtT[:, q * P : q * P + rq], pt[:, :rq])

        # main streaming loop
        emb_r = emb[:].rearrange("(ch t p) d -> ch p t d", t=chunk, p=P)
        # two accumulators in different PSUM banks so consecutive matmuls
        # don't serialize on same-bank accumulate drains
        psum_scA = pacc.tile([C, D + 1], F32)
        psum_scB = pacc.tile([C, D + 1], F32)
        ssq_a = consts.tile([P, 1], F32)
        ssq_b = consts.tile([P, 1], F32)
        ssq = [ssq_a, ssq_b]
        scratch = consts.tile([P, chunk * D], F32)
        for ch in range(nch):
            dma_eng = nc.sync if ch % 2 == 0 else nc.scalar
            ck = chunks.tile([P, chunk, D], F32, tag="ck")
            dma_eng.dma_start(ck[:], emb_r[ch])
            ckb = chunksb.tile([P, chunk, D + 1], BF16, tag="ckb")
            nc.vector.tensor_copy(ckb[:, :, 0:D], ck[:])
            nc.vector.memset(ckb[:, :, D], 1.0)
            # fp32 sum-of-squares for the whole chunk in one ACT op
            sq = ohp.tile([P, 1], F32, tag="sq")
            nc.scalar.activation(
                scratch[:],
                ck[:].rearrange("p t d -> p (t d)"),
                AF.Square,
                bias=zero_p[:],
                accum_out=sq[:],
            )
            if ch == 0:
                nc.vector.tensor_copy(ssq[1][:], sq[:])
            else:
                nc.vector.tensor_add(ssq[(ch + 1) % 2][:], ssq[ch % 2][:], sq[:])
            for tb in range(chunk // OHB):
                i0 = ch * chunk + tb * OHB
                oh8 = ohp.tile([P, OHB, C], BF16, tag="oh8")
                cols = tgtT[:, i0 : i0 + OHB]
                bcast = bass.AP(
                    tensor=cols.tensor,
                    offset=cols.offset,
                    ap=[cols.ap[0], cols.ap[1], [0, C]],
                )
                nc.vector.tensor_tensor(oh8[:], iota_t[:], bcast, OP.is_equal)
                for j in range(OHB):
                    i = i0 + j
                    t = tb * OHB + j
                    acc = psum_scA if i % 2 == 0 else psum_scB
                    nc.tensor.matmul(
                        acc[:],
                        oh8[:, j, :],
                        ckb[:, t, :],
                        start=(i < 2),
                        stop=(i >= T - 2),
                    )
        ssq_fin = ssq[nch % 2]

        # reduce ssq column to a scalar on PE, pack local partials [C, PKW]:
        # [:, 0:129] = sums|counts, [0,129] = ssq
        ssq_ps = ptr.tile([P, P], F32, tag="fin_ps")
        nc.tensor.matmul(
            ssq_ps[0:1, 0:1], ones128[:], ssq_fin[:], start=True, stop=True
        )
        pk = finp.tile([C, PKW], F32)
        nc.vector.memset(pk[:], 0.0)
        nc.scalar.copy(pk[:, 0 : D + 1], psum_scA[:])
        nc.vector.tensor_add(pk[:, 0 : D + 1], pk[:, 0 : D + 1], psum_scB[:])
        nc.scalar.copy(pk[0:1, 129:130], ssq_ps[0:1, 0:1])

        # all-gather partials, reduce over ranks
        cc_in = dram.tile([C, PKW], F32)
        cc_out = dram.tile([NCORES * C, PKW], F32)
        nc.sync.dma_start(cc_in[:], pk[:])
        nc.gpsimd.collective_compute(
            "AllGather",
            OP.bypass,
            replica_groups=[list(range(NCORES))],
            ins=[cc_in[:].opt()],
            outs=[cc_out[:].opt()],
        )
        g = finp.tile([C, NCORES, PKW], F32)
        nc.sync.dma_start(g[:], cc_out[:].rearrange("(r p) f -> p r f", p=C))
        h1 = [
            finp.tile([C, PKW], F32, tag=f"h1_{j}", name=f"h1_{j}") for j in range(4)
        ]
        for j in range(4):
            nc.vector.tensor_add(h1[j][:], g[:, 2 * j, :], g[:, 2 * j + 1, :])
        h2a = finp.tile([C, PKW], F32)
        h2b = finp.tile([C, PKW], F32)
        nc.vector.tensor_add(h2a[:], h1[0][:], h1[1][:])
        nc.vector.tensor_add(h2b[:], h1[2][:], h1[3][:])
        tot = finp.tile([C, PKW], F32)
        nc.vector.tensor_add(tot[:], h2a[:], h2b[:])

        S = tot[:, 0:D]
        cnt = tot[:, D : D + 1]
        ssq_g = tot[0:1, 129:130]

        # centers = S / (cnt + eps)
        cnt_eps = finp.tile([C, 1], F32)
        nc.vector.tensor_scalar_add(cnt_eps[:], cnt, EPS)
        rc = finp.tile([C, 1], F32)
        nc.vector.reciprocal(rc[:], cnt_eps[:])
        centers = finp.tile([C, D], F32)
        nc.vector.tensor_scalar(
            out=centers[:], in0=S, scalar1=rc[:], scalar2=None, op0=OP.mult
        )

        # l2 pieces: dot1 = <S,c>, rn = ||c||^2, qcol = cnt*rn - 2*dot1
        scr1 = finp.tile([C, D], F32)
        dot1 = finp.tile([C, 1], F32)
        nc.vector.tensor_mul(scr1[:], S, centers[:])
        nc.vector.tensor_reduce(dot1[:], scr1[:], mybir.AxisListType.X, OP.add)
        scr2 = finp.tile([C, D], F32)
        rn = finp.tile([C, 1], F32)
        nc.vector.tensor_mul(scr2[:], centers[:], centers[:])
        nc.vector.tensor_reduce(rn[:], scr2[:], mybir.AxisListType.X, OP.add)
        q1 = finp.tile([C, 1], F32)
        nc.vector.tensor_tensor(q1[:], cnt, rn[:], OP.mult)
        qcol = finp.tile([C, 1], F32)
        nc.vector.scalar_tensor_tensor(
            out=qcol[:], in0=dot1[:], scalar=-2.0, in1=q1[:],
            op0=OP.mult, op1=OP.add,
        )

        # pairwise: sqdist = rn_a + rn_b - 2 c.c^T via PSUM accumulation
        pt2 = ptr.tile([P, P], F32, tag="fin_ps")
        nc.tensor.transpose(pt2[:, 0:C], centers[:], eye_t[0:C, 0:C])
        ct = finp.tile([P, C], F32)
        nc.scalar.copy(ct[:], pt2[:, 0:C])
        ctm2 = finp.tile([P, C], F32)
        nc.scalar.mul(ctm2[:], ct[:], -2.0)
        csq = finp.tile([P, C], F32)
        nc.scalar.activation(csq[:], ct[:], AF.Square, bias=zero_p[:])
        prn = ptr.tile([P, P], F32, tag="fin_ps")
        nc.tensor.matmul(prn[0:1, 0:C], ones128[:], csq[:], start=True, stop=True)
        rn_row = finp.tile([1, C], F32)
        nc.scalar.copy(rn_row[:], prn[0:1, 0:C])

        pd = ptr.tile([P, P], F32, tag="fin_ps")
        nc.tensor.matmul(pd[0:C, 0:C], rn_row[:], ones_row[:], start=True, stop=False)
        nc.tensor.matmul(pd[0:C, 0:C], ones_row[:], rn_row[:], start=False, stop=False)
        nc.tensor.matmul(pd[0:C, 0:C], ctm2[:], ct[:], start=False, stop=True)

        u = finp.tile([C, C], F32)
        nc.scalar.activation(u[:], pd[0:C, 0:C], AF.Sqrt, bias=eps_c[:], scale=1.0)
        v = finp.tile([C, C], F32)
        nc.scalar.activation(v[:], u[:], AF.Relu, bias=mar_c[:], scale=-1.0)
        vm = finp.tile([C, C], F32)
        nc.vector.tensor_mul(vm[:], v[:], triu_t[:])
        scr3 = finp.tile([C, C], F32)
        cc_col = finp.tile([C, 1], F32)
        nc.vector.tensor_mul(scr3[:], vm[:], vm[:])
        nc.vector.tensor_reduce(cc_col[:], scr3[:], mybir.AxisListType.X, OP.add)

        # final scalar: invN*(ssq + sum qcol) + ccs*sum cc_col
        pf = ptr.tile([P, P], F32, tag="fin_ps")
        nc.tensor.matmul(pf[0:1, 0:1], sc_invN[0:1, :], ssq_g, start=True, stop=False)
        nc.tensor.matmul(pf[0:1, 0:1], sc_invN[0:C, :], qcol[:], start=False, stop=False)
        nc.tensor.matmul(pf[0:1, 0:1], sc_ccs[:], cc_col[:], start=False, stop=True)
        res = finp.tile([1, 1], F32)
        nc.scalar.copy(res[:], pf[0:1, 0:1])
        nc.sync.dma_start(out[:], res[:])

    nc.compile()
    return nc


_NC_CACHE = {}


def _get_nc(nloc, chunk):
    key = (nloc, chunk)
    if key not in _NC_CACHE:
        _NC_CACHE[key] = _build(nloc, chunk)
    return _NC_CACHE[key]


def _consts():
    import ml_dtypes

    iota_c = np.tile(np.arange(C, dtype=np.float32), (P, OHB, 1)).astype(
        ml_dtypes.bfloat16
    )
    eye = np.eye(P, dtype=np.float32)
    triu = np.triu(np.ones((C, C), dtype=np.float32), k=1)
    return iota_c, eye, triu


def _run(embeddings, target, nloc=NLOC, chunk=16, trace=False):
    emb = np.ascontiguousarray(np.asarray(embeddings, dtype=np.float32))
    tgt = np.ascontiguousarray(np.asarray(target).astype(np.int32))
    n = emb.shape[0]
    assert n == nloc * NCORES and emb.shape[1] == D
    nc = _get_nc(nloc, chunk)
    iota_c, eye, triu = _consts()
    in_maps = []
    for k in range(NCORES):
        in_maps.append(
            {
                "emb": emb[k * nloc : (k + 1) * nloc],
                "tgt": tgt[k * nloc : (k + 1) * nloc],
                "iota_c": iota_c,
                "eye": eye,
                "triu": triu,
            }
        )
    r = run_bass_kernel_spmd(nc, in_maps, core_ids=list(range(NCORES)), trace=trace)
    val = r.results[0]["out"].reshape(())
    return np.asarray(val, dtype=np.float32), r


def kernel(embeddings, target):
    out, _ = _run(embeddings, target)
    return out


# revision 18
# speedup vs baseline: 1.0760x; 1.0760x over previous
"""CentroidLoss Trainium2 kernel (Bass/Tile, 8-core SPMD).

Math: the loss decomposes so embeddings are streamed exactly once.
  counts_c = #{n: t_n = c};  S_c = sum_{n in c} e_n;  centers c_c = S_c/(counts_c+eps)
  l2  = mean_n ||e_n - c_{t_n}||^2
      = ( sum_n ||e_n||^2 - 2 sum_c S_c.c_c + sum_c counts_c ||c_c||^2 ) / N
  cc  = sum_{a<b} relu(margin - sqrt(||c_a-c_b||^2 + eps))^2 * (C-1)/(2C)

Per core: stream N/8 rows once (1MB+ DMA chunks, issued ahead of everything
else on the sync ring); per-class sums+counts via one-hot matmuls (bf16
operands, fp32 PSUM accumulation, two PSUM banks round-robin); one-hots for 8
row-tiles are built with a single DVE is_equal against a stride-0-broadcast
target column block; the dominant sum-of-squares term runs in fp32 via one ACT
Square+accumulate per chunk.  Partials are packed into bf16 (sums rounded —
negligible after averaging; counts and ssq split losslessly into bf16 value +
integer residual), AllGathered across the 8 cores, reduced, and the
closed-form finale runs replicated on DVE/PE (ACT only for the sqrt).
"""

import numpy as np

try:
    import concourse.bass as bass
except ImportError:  # fallback when PYTHONPATH doesn't carry the repo
    import sys

    for _p in ("/opt/trn_rl_repo", "/root/.axon_site/_ro/trn_rl_repo"):
        if _p not in sys.path:
            sys.path.insert(0, _p)
    import concourse.bass as bass

from contextlib import ExitStack

import concourse.bacc as bacc
import concourse.tile as tile
from concourse import mybir
from concourse.bass_utils import run_bass_kernel_spmd

F32 = mybir.dt.float32
BF16 = mybir.dt.bfloat16
I32 = mybir.dt.int32
OP = mybir.AluOpType
AF = mybir.ActivationFunctionType

N_FULL = 262144
D = 128
C = 64
NCORES = 8
P = 128
MARGIN = 1.4
EPS = 1e-6

NLOC = N_FULL // NCORES  # rows per core
OHB = 8  # row-tiles of one-hot built per DVE instruction


def _build(nloc, chunk):
    """Trace the SPMD kernel for `nloc` rows/core, `chunk` row-tiles per DMA."""
    T = nloc // P  # 128-row tiles per core
    assert nloc % P == 0 and T % chunk == 0 and chunk % OHB == 0
    nch = T // chunk
    nq = (T + P - 1) // P  # groups of <=128 tiles for the target transpose
    invN = 1.0 / (nloc * NCORES)
    ccs = (C - 1) / (2.0 * C)
    # packed partial width: 0..127 sums(bf16), 128/129 counts hi+resid,
    # 130/131 ssq hi+resid (row 0)
    PKW = 132

    nc = bacc.Bacc("TRN2", target_bir_lowering=False, debug=False, num_devices=NCORES)
    emb = nc.dram_tensor("emb", [nloc, D], F32, kind="ExternalInput")
    tgt = nc.dram_tensor("tgt", [nloc], I32, kind="ExternalInput")
    iota_in = nc.dram_tensor("iota_c", [P, OHB, C], BF16, kind="ExternalInput")
    eye_in = nc.dram_tensor("eye", [P, P], F32, kind="ExternalInput")
    triu_in = nc.dram_tensor("triu", [C, C], F32, kind="ExternalInput")
    out = nc.dram_tensor("out", [1, 1], F32, kind="ExternalOutput")

    with tile.TileContext(nc) as tc, ExitStack() as ctx:
        consts = ctx.enter_context(tc.tile_pool(name="consts", bufs=1))
        chunks = ctx.enter_context(tc.tile_pool(name="chunks", bufs=4))
        chunksb = ctx.enter_context(tc.tile_pool(name="chunksb", bufs=4))
        ohp = ctx.enter_context(tc.tile_pool(name="ohp", bufs=6))
        pacc = ctx.enter_context(tc.tile_pool(name="pacc", bufs=1, space="PSUM"))
        ptr = ctx.enter_context(tc.tile_pool(name="ptr", bufs=2, space="PSUM"))
        dram = ctx.enter_context(tc.tile_pool(name="dram", bufs=1, space="DRAM"))
        finp = ctx.enter_context(tc.tile_pool(name="finp", bufs=1))

        # issue all embedding-chunk DMAs first so the sync ring starts moving
        # data immediately; slot waits (bufs=4) gate the later ones naturally
        emb_r = emb[:].rearrange("(ch t p) d -> ch p t d", t=chunk, p=P)
        ck_tiles = []
        for ch in range(nch):
            ck = chunks.tile([P, chunk, D], F32, tag="ck", name=f"ck{ch}")
            nc.sync.dma_start(ck[:], emb_r[ch])
            ck_tiles.append(ck)

        # constants + targets ride the (otherwise idle) scalar HWDGE ring
        iota_t = consts.tile([P, OHB, C], BF16)
        nc.scalar.dma_start(iota_t[:], iota_in[:])
        eye_t = consts.tile([P, P], F32)
        nc.scalar.dma_start(eye_t[:], eye_in[:])
        eye16 = consts.tile([P, P], BF16)
        nc.vector.tensor_copy(eye16[:], eye_t[:])
        triu_t = consts.tile([C, C], F32)
        nc.scalar.dma_start(triu_t[:], triu_in[:])
        ones128 = consts.tile([P, 1], F32)
        nc.vector.memset(ones128[:], 1.0)
        ones_row = consts.tile([1, C], F32)
        nc.vector.memset(ones_row[:], 1.0)
        sc_invN = consts.tile([P, 1], F32)
        nc.vector.memset(sc_invN[:], invN)
        sc_ccs = consts.tile([C, 1], F32)
        nc.vector.memset(sc_ccs[:], ccs)
        eps_c = consts.tile([C, 1], F32)
        nc.vector.memset(eps_c[:], EPS)
        zero_p = consts.tile([P, 1], F32)
        nc.vector.memset(zero_p[:], 0.0)

        # targets -> tgtT[P, T]: column i holds the 128 targets of row-tile i
        tgt2 = tgt[:].rearrange("(t c) -> t c", c=P)  # [T, P]
        tgtT = consts.tile([P, T], BF16)
        for q in range(nq):
            rq = min(P, T - q * P)
            ti = consts.tile([P, P], I32, tag="tgt_i32")
            tf = consts.tile([P, P], BF16, tag="tgt_bf")
            nc.scalar.dma_start(ti[:rq, :], tgt2[q * P : q * P + rq, :])
            nc.vector.tensor_copy(tf[:rq, :], ti[:rq, :])
            pt = ptr.tile([P, P], BF16, tag="pt")
            nc.tensor.transpose(pt[:, :rq], tf[:rq, :], eye16[:rq, :rq])
            nc.scalar.copy(tgtT[:, q * P : q * P + rq], pt[:, :rq])

        # main streaming loop; two PSUM accumulators so consecutive matmuls
        # alternate banks instead of serializing on same-bank accumulates
        psum_scA = pacc.tile([C, D + 1], F32)
        psum_scB = pacc.tile([C, D + 1], F32)
        ssq_a = consts.tile([P, 1], F32)
        ssq_b = consts.tile([P, 1], F32)
        ssq = [ssq_a, ssq_b]
        scratch = consts.tile([P, chunk * D], F32)
        for ch in range(nch):
            ck = ck_tiles[ch]
            ckb = chunksb.tile([P, chunk, D + 1], BF16, tag="ckb")
            nc.vector.tensor_copy(ckb[:, :, 0:D], ck[:])
            nc.vector.memset(ckb[:, :, D], 1.0)
            # fp32 sum-of-squares for the whole chunk in one ACT op
            sq = ohp.tile([P, 1], F32, tag="sq")
            nc.scalar.activation(
                scratch[:],
                ck[:].rearrange("p t d -> p (t d)"),
                AF.Square,
                bias=zero_p[:],
                accum_out=sq[:],
            )
            if ch == 0:
                nc.vector.tensor_copy(ssq[1][:], sq[:])
            else:
                nc.vector.tensor_add(ssq[(ch + 1) % 2][:], ssq[ch % 2][:], sq[:])
            for tb in range(chunk // OHB):
                i0 = ch * chunk + tb * OHB
                oh8 = ohp.tile([P, OHB, C], BF16, tag="oh8")
                cols = tgtT[:, i0 : i0 + OHB]
                bcast = bass.AP(
                    tensor=cols.tensor,
                    offset=cols.offset,
                    ap=[cols.ap[0], cols.ap[1], [0, C]],
                )
                nc.vector.tensor_tensor(oh8[:], iota_t[:], bcast, OP.is_equal)
                for j in range(OHB):
                    i = i0 + j
                    t = tb * OHB + j
                    acc = psum_scA if i % 2 == 0 else psum_scB
                    nc.tensor.matmul(
                        acc[:],
                        oh8[:, j, :],
                        ckb[:, t, :],
                        start=(i < 2),
                        stop=(i >= T - 2),
                    )
        ssq_fin = ssq[nch % 2]

        # reduce ssq column to a scalar on PE, assemble local partials
        ssq_ps = ptr.tile([P, P], F32, tag="fin_ps")
        nc.tensor.matmul(
            ssq_ps[0:1, 0:1], ones128[:], ssq_fin[:], start=True, stop=True
        )
        pk = finp.tile([C, PKW], F32)
        nc.vector.memset(pk[:], 0.0)
        nc.vector.tensor_copy(pk[:, 0 : D + 1], psum_scA[:])
        nc.vector.tensor_add(pk[:, 0 : D + 1], pk[:, 0 : D + 1], psum_scB[:])
        nc.vector.tensor_copy(pk[0:1, 130:131], ssq_ps[0:1, 0:1])

        # pack to bf16: sums rounded; counts/ssq split value+residual (exact)
        pkb = finp.tile([C, PKW], BF16)
        nc.vector.memset(pkb[:], 0.0)
        nc.vector.tensor_copy(pkb[:, 0:D], pk[:, 0:D])
        nc.vector.tensor_copy(pkb[:, D : D + 1], pk[:, D : D + 1])  # cnt hi (bf16)
        chi32 = finp.tile([C, 1], F32)
        nc.vector.tensor_copy(chi32[:], pkb[:, D : D + 1])
        nc.vector.tensor_sub(chi32[:], pk[:, D : D + 1], chi32[:])  # resid
        nc.vector.tensor_copy(pkb[:, 129:130], chi32[:])
        nc.vector.tensor_copy(pkb[0:1, 130:131], pk[0:1, 130:131])  # ssq hi
        shi32 = finp.tile([1, 1], F32)
        nc.vector.tensor_copy(shi32[:], pkb[0:1, 130:131])
        nc.vector.tensor_sub(shi32[:], pk[0:1, 130:131], shi32[:])
        nc.vector.tensor_copy(pkb[0:1, 131:132], shi32[:])

        # all-gather bf16 partials, reduce over ranks in fp32
        cc_in = dram.tile([C, PKW], BF16)
        cc_out = dram.tile([NCORES * C, PKW], BF16)
        nc.sync.dma_start(cc_in[:], pkb[:])
        nc.gpsimd.collective_compute(
            "AllGather",
            OP.bypass,
            replica_groups=[list(range(NCORES))],
            ins=[cc_in[:].opt()],
            outs=[cc_out[:].opt()],
        )
        g = finp.tile([C, NCORES, PKW], BF16)
        nc.sync.dma_start(g[:], cc_out[:].rearrange("(r p) f -> p r f", p=C))
        h1 = [
            finp.tile([C, PKW], F32, tag=f"h1_{j}", name=f"h1_{j}") for j in range(4)
        ]
        for j in range(4):
            nc.vector.tensor_add(h1[j][:], g[:, 2 * j, :], g[:, 2 * j + 1, :])
        h2a = finp.tile([C, PKW], F32)
        h2b = finp.tile([C, PKW], F32)
        nc.vector.tensor_add(h2a[:], h1[0][:], h1[1][:])
        nc.vector.tensor_add(h2b[:], h1[2][:], h1[3][:])
        tot = finp.tile([C, PKW], F32)
        nc.vector.tensor_add(tot[:], h2a[:], h2b[:])

        S = tot[:, 0:D]
        cnt_col = finp.tile([C, 1], F32)
        nc.vector.tensor_add(cnt_col[:], tot[:, D : D + 1], tot[:, 129:130])
        ssq_t = finp.tile([1, 1], F32)
        nc.vector.tensor_add(ssq_t[:], tot[0:1, 130:131], tot[0:1, 131:132])

        # centers = S / (cnt + eps)
        cnt_eps = finp.tile([C, 1], F32)
        nc.vector.tensor_scalar_add(cnt_eps[:], cnt_col[:], EPS)
        rc = finp.tile([C, 1], F32)
        nc.vector.reciprocal(rc[:], cnt_eps[:])
        centers = finp.tile([C, D], F32)
        nc.vector.tensor_scalar(
            out=centers[:], in0=S, scalar1=rc[:], scalar2=None, op0=OP.mult
        )

        # l2 pieces: dot1 = <S,c>, rn = ||c||^2, qcol = cnt*rn - 2*dot1
        scr1 = finp.tile([C, D], F32)
        dot1 = finp.tile([C, 1], F32)
        nc.vector.tensor_mul(scr1[:], S, centers[:])
        nc.vector.tensor_reduce(dot1[:], scr1[:], mybir.AxisListType.X, OP.add)
        scr2 = finp.tile([C, D], F32)
        rn = finp.tile([C, 1], F32)
        nc.vector.tensor_mul(scr2[:], centers[:], centers[:])
        nc.vector.tensor_reduce(rn[:], scr2[:], mybir.AxisListType.X, OP.add)
        q1 = finp.tile([C, 1], F32)
        nc.vector.tensor_tensor(q1[:], cnt_col[:], rn[:], OP.mult)
        qcol = finp.tile([C, 1], F32)
        nc.vector.scalar_tensor_tensor(
            out=qcol[:], in0=dot1[:], scalar=-2.0, in1=q1[:],
            op0=OP.mult, op1=OP.add,
        )

        # pairwise: sqdist = rn_a + rn_b - 2 c.c^T via PSUM accumulation
        pt2 = ptr.tile([P, P], F32, tag="fin_ps")
        nc.tensor.transpose(pt2[:, 0:C], centers[:], eye_t[0:C, 0:C])
        ct = finp.tile([P, C], F32)
        nc.vector.tensor_copy(ct[:], pt2[:, 0:C])
        ctm2 = finp.tile([P, C], F32)
        nc.vector.tensor_scalar_mul(ctm2[:], ct[:], -2.0)
        csq = finp.tile([P, C], F32)
        nc.vector.tensor_mul(csq[:], ct[:], ct[:])
        prn = ptr.tile([P, P], F32, tag="fin_ps")
        nc.tensor.matmul(prn[0:1, 0:C], ones128[:], csq[:], start=True, stop=True)
        rn_row = finp.tile([1, C], F32)
        nc.vector.tensor_copy(rn_row[:], prn[0:1, 0:C])

        pd = ptr.tile([P, P], F32, tag="fin_ps")
        nc.tensor.matmul(pd[0:C, 0:C], rn_row[:], ones_row[:], start=True, stop=False)
        nc.tensor.matmul(pd[0:C, 0:C], ones_row[:], rn_row[:], start=False, stop=False)
        nc.tensor.matmul(pd[0:C, 0:C], ctm2[:], ct[:], start=False, stop=True)

        u = finp.tile([C, C], F32)
        nc.scalar.activation(u[:], pd[0:C, 0:C], AF.Sqrt, bias=eps_c[:], scale=1.0)
        v = finp.tile([C, C], F32)
        nc.vector.tensor_scalar(
            out=v[:], in0=u[:], scalar1=-1.0, scalar2=MARGIN, op0=OP.mult, op1=OP.add
        )
        v2 = finp.tile([C, C], F32)
        nc.vector.tensor_scalar_max(v2[:], v[:], 0.0)
        vm = finp.tile([C, C], F32)
        nc.vector.tensor_mul(vm[:], v2[:], triu_t[:])
        scr3 = finp.tile([C, C], F32)
        cc_col = finp.tile([C, 1], F32)
        nc.vector.tensor_mul(scr3[:], vm[:], vm[:])
        nc.vector.tensor_reduce(cc_col[:], scr3[:], mybir.AxisListType.X, OP.add)

        # final scalar: invN*(ssq + sum qcol) + ccs*sum cc_col
        pf = ptr.tile([P, P], F32, tag="fin_ps")
        nc.tensor.matmul(pf[0:1, 0:1], sc_invN[0:1, :], ssq_t[:], start=True, stop=False)
        nc.tensor.matmul(pf[0:1, 0:1], sc_invN[0:C, :], qcol[:], start=False, stop=False)
        nc.tensor.matmul(pf[0:1, 0:1], sc_ccs[:], cc_col[:], start=False, stop=True)
        res = finp.tile([1, 1], F32)
        nc.vector.tensor_copy(res[:], pf[0:1, 0:1])
        nc.sync.dma_start(out[:], res[:])

    nc.compile()
    return nc


_NC_CACHE = {}


def _get_nc(nloc, chunk):
    key = (nloc, chunk)
    if key not in _NC_CACHE:
        _NC_CACHE[key] = _build(nloc, chunk)
    return _NC_CACHE[key]


def _consts():
    import ml_dtypes

    iota_c = np.tile(np.arange(C, dtype=np.float32), (P, OHB, 1)).astype(
        ml_dtypes.bfloat16
    )
    eye = np.eye(P, dtype=np.float32)
    triu = np.triu(np.ones((C, C), dtype=np.float32), k=1)
    return iota_c, eye, triu


def _run(embeddings, target, nloc=NLOC, chunk=16, trace=False):
    emb = np.ascontiguousarray(np.asarray(embeddings, dtype=np.float32))
    tgt = np.ascontiguousarray(np.asarray(target).astype(np.int32))
    n = emb.shape[0]
    assert n == nloc * NCORES and emb.shape[1] == D
    nc = _get_nc(nloc, chunk)
    iota_c, eye, triu = _consts()
    in_maps = []
    for k in range(NCORES):
        in_maps.append(
            {
                "emb": emb[k * nloc : (k + 1) * nloc],
                "tgt": tgt[k * nloc : (k + 1) * nloc],
                "iota_c": iota_c,
                "eye": eye,
                "triu": triu,
            }
        )
    r = run_bass_kernel_spmd(nc, in_maps, core_ids=list(range(NCORES)), trace=trace)
    val = r.results[0]["out"].reshape(())
    return np.asarray(val, dtype=np.float32), r


def kernel(embeddings, target):
    out, _ = _run(embeddings, target)
    return out
